# revision 19
# baseline (speedup 1.0000x reference)
"""Multi-head self-attention (B=4, S=2048, D=1024, H=16) on 8 trn2 NeuronCores.

Sharding: tensor-parallel over heads - each core computes 2 heads (a 128-dim
slice of the q/k/v projection space) for all 4 batches.

Per-core dataflow (all matmuls in float32r - full PE rate, ~2e-4 rel err):
  - host pre-transposes x -> xT [B, D, S] and weight slices.
  - projections produce QT/KT [dims=128, S] directly (contract over D).
  - scores are computed k-major: S^T[k, q] = KT^T-slice @ QT-slice, so the
    exp'd tiles E^T are directly the moving operand of the context matmul
    (contract over k) with V in natural [k, d] layout (from a PE transpose
    of VT).  A ones-column appended to V yields the softmax denominators in
    the same matmul.  exp on ScalarE (<=2ulp); normalization via a
    reciprocal row broadcast across partitions with a K=1 PE matmul and one
    DVE multiply pass.
  - attn_weights are written k-major (attnT [h, b, k, q]); the host returns
    a transposed numpy view (values identical, strides swapped).
  - out-projection partials [B, S, D] are summed on the host (unshard).
"""

import sys

sys.path.insert(0, "/opt/trn_rl_repo")

import numpy as np  # noqa: E402

import concourse.bacc as bacc  # noqa: E402
import concourse.bass as bass  # noqa: E402
import concourse.mybir as mybir  # noqa: E402
import concourse.tile as tile  # noqa: E402
import concourse.tile_utils as tile_utils  # noqa: E402

tile_utils.max_sbuf_usage = 208 * 1024  # 208KB usable on trn2; stale 192 cap

from concourse.bass_utils import run_bass_kernel_spmd  # noqa: E402

F32 = mybir.dt.float32
F32R = mybir.dt.float32r
EXP = mybir.ActivationFunctionType.Exp

N_CORES = 8
HEADS_PER_CORE = 2
HEAD_DIM = 64

FULL_CFG = dict(B=4, S=2048, D=1024)


def build_program(B, S, D, repeat=1):
    """Build the per-core SPMD program (identical on all cores)."""
    NCC = D // 128          # c-chunks (contraction tiles) in d_model
    NKT = S // 128          # k tiles
    QSW = min(512, S)       # q strip width
    NQS = S // QSW          # q strips
    KG = min(2, NKT)        # k-tiles per probs-multiply/DMA group
    NKG = NKT // KG
    OW = min(512, D)        # out-proj N tile width
    NOW = D // OW
    NST = S // 128          # s tiles for out-proj
    scale = 1.0 / np.sqrt(HEAD_DIM)

    nc = bacc.Bacc(None, target_bir_lowering=False, num_swdge_queues=4)

    xT_d = nc.declare_dram_parameter("xT", [B, D, S], F32R, isOutput=False)
    wT_d = nc.declare_dram_parameter("wT", [D, 384], F32R, isOutput=False)
    woT_d = nc.declare_dram_parameter("woT", [64, 2, D], F32R, isOutput=False)
    bqkv_d = nc.declare_dram_parameter("bqkv", [128, 3], F32, isOutput=False)
    attnT_d = nc.declare_dram_parameter(
        "attnT", [HEADS_PER_CORE, B, S, S], F32, isOutput=True
    )
    outp_d = nc.declare_dram_parameter("outp", [B, S, D], F32, isOutput=True)

    with tile.TileContext(nc) as tc:
        with (
            tc.tile_pool(name="cst", bufs=1) as cst,
            tc.tile_pool(name="sbA", bufs=1) as sbA,
            tc.tile_pool(name="sbB", bufs=2) as sbB,
            tc.tile_pool(name="sbC", bufs=2) as sbC,
            tc.tile_pool(name="psA", bufs=1, space="PSUM") as psA,
            tc.tile_pool(name="psB", bufs=2, space="PSUM") as psB,
        ):
            # --- constants ---
            wT_sb = cst.tile([128, NCC, 384], F32R, tag="w")
            nc.sync.dma_start(
                wT_sb[:], wT_d.rearrange("(cc p) w -> p cc w", p=128)
            )
            woT_sb = cst.tile([64, 2, D], F32R, tag="wo")
            nc.sync.dma_start(woT_sb[:], woT_d[:])
            bias_sb = cst.tile([128, 3], F32, tag="b")
            nc.sync.dma_start(bias_sb[:], bqkv_d[:])
            ones_sb = cst.tile([128, 128], F32, tag="ones")
            nc.gpsimd.memset(ones_sb[:], 1.0)
            c32_sb = cst.tile([128, 128], F32, tag="c32")
            nc.gpsimd.memset(c32_sb[:], 1.0 / 8.0)
            # ident64: two stacked 64x64 identities (rows 0-63 and 64-127)
            ident64 = cst.tile([128, 64], F32, tag="id")
            nc.gpsimd.memset(ident64[:], 0.0)
            for hh in range(2):
                nc.gpsimd.affine_select(
                    out=ident64[hh * 64 : (hh + 1) * 64, :],
                    in_=ident64[hh * 64 : (hh + 1) * 64, :],
                    compare_op=mybir.AluOpType.not_equal,
                    fill=1.0,
                    base=0,
                    pattern=[[-1, 64]],
                    channel_multiplier=1,
                )

            for b in [bb for _ in range(repeat) for bb in range(B)]:
                # ---------------- projections for batch b ----------------
                QT_sb = sbC.tile([128, S], F32R, tag="qt")
                KT_sb = sbC.tile([128, S], F32R, tag="kt")
                VT_sb = sbA.tile([128, S], F32, tag="vt")
                V_sb = sbC.tile([128, 2, NKT, 72], F32R, tag="v")
                ctxT_sb = sbA.tile([64, 2, S], F32R, tag="ctx_sb")
                # ones columns (64..71) of each V tile -> softmax denominators
                for h in range(2):
                    ones_in = bass.AP(
                        ones_sb[:].tensor, ones_sb[:].offset,
                        [ones_sb[:].ap[0], [0, NKT], [1, 8]],
                    )
                    nc.vector.tensor_copy(V_sb[:, h, :, 64:72], ones_in)
                xT_b = xT_d[b].rearrange("(cc p) s -> p cc s", p=128)
                for q in range(NQS):
                    xq = sbA.tile([128, NCC, QSW], F32R, tag="xq")
                    nc.sync.dma_start(
                        xq[:], xT_b[:, :, q * QSW : (q + 1) * QSW]
                    )
                    for t, dst in enumerate((QT_sb, KT_sb, VT_sb)):
                        pp = psB.tile([128, QSW], F32, tag="proj")
                        for cc in range(NCC):
                            nc.tensor.matmul(
                                pp[:],
                                wT_sb[:, cc, t * 128 : (t + 1) * 128],
                                xq[:, cc, :],
                                start=(cc == 0),
                                stop=(cc == NCC - 1),
                            )
                        nc.vector.tensor_scalar_add(
                            dst[:, q * QSW : (q + 1) * QSW],
                            pp[:],
                            bias_sb[:, t : t + 1],
                        )
                # V = VT^T per head (cols 0..63 of each V k-tile)
                for h in range(2):
                    for g in range(NKT // 4):
                        tp = psB.tile([128, 8, 64], F32, tag="proj")
                        for j in range(4):
                            kt = g * 4 + j
                            nc.tensor.transpose(
                                tp[:, j, :],
                                VT_sb[h * 64 : (h + 1) * 64, kt * 128 : (kt + 1) * 128],
                                ident64[h * 64 : (h + 1) * 64, :],
                            )
                        nc.vector.tensor_copy(
                            V_sb[:, h, g * 4 : (g + 1) * 4, 0:64], tp[:, 0:4, :]
                        )

                # ------ attention + interleaved out-proj for batch b ------
                for qs in range(NQS):
                    qsl = slice(qs * QSW, (qs + 1) * QSW)
                    for h in range(2):
                        hs = slice(h * 64, (h + 1) * 64)
                        E = sbB.tile([128, NKT, QSW], F32R, tag="e")
                        for kt in range(NKT):
                            sc = psB.tile([128, QSW], F32, tag="score")
                            nc.tensor.matmul(
                                sc[:],
                                KT_sb[hs, kt * 128 : (kt + 1) * 128],
                                QT_sb[hs, qsl],
                                start=True,
                                stop=True,
                            )
                            nc.scalar.activation(
                                E[:, kt, :], sc[:], EXP, bias=0.0, scale=float(scale)
                            )
                        ctx = psB.tile([128, QSW], F32, tag="ctx")
                        for kt in range(NKT):
                            nc.tensor.matmul(
                                ctx[0:72, :],
                                V_sb[:, h, kt, :],
                                E[:, kt, :],
                                start=(kt == 0),
                                stop=(kt == NKT - 1),
                            )
                        # sums row (row 64 of ctx psum) -> partition 0 via
                        # SBUF-SBUF DMA, reciprocal there, then K=1 base-0
                        # matmul broadcasts it across all 128 partitions
                        # (K<64 matmuls at base 64 misread their operands on
                        # HW, so everything here runs at base 0)
                        srow = sbB.tile([65, QSW], F32, tag="sr")
                        nc.vector.tensor_copy(srow[64:65, :], ctx[64:65, :])
                        srow0 = sbB.tile([1, QSW], F32, tag="sr0")
                        nc.sync.dma_start(srow0[0:1, :], srow[64:65, :])
                        rrow = sbB.tile([1, QSW], F32, tag="rc")
                        nc.vector.reciprocal_approx_fast(
                            rrow[0:1, :], srow0[0:1, :]
                        )
                        bc = psA.tile([128, QSW], F32, tag="bcast")
                        nc.tensor.matmul(
                            bc[:],
                            ones_sb[0:1, :],
                            rrow[0:1, :],
                            start=True,
                            stop=True,
                        )
                        bc_sb = sbB.tile([128, QSW], F32, tag="bc")
                        nc.vector.tensor_copy(bc_sb[:], bc[:])
                        # normalized context -> ctxT_sb (f32r for out-proj)
                        nc.vector.tensor_mul(
                            ctxT_sb[:, h, qsl], ctx[0:64, :], bc_sb[0:64, :]
                        )
                        # normalized probs -> DRAM (k-major), in groups of KG
                        attn_hb = attnT_d[h, b].rearrange(
                            "(g p) q -> p g q", p=128
                        )
                        for g in range(NKG):
                            pr = sbB.tile([128, KG, QSW], F32, tag="pr")
                            bca = bc_sb[:]
                            in1 = bass.AP(
                                bca.tensor,
                                bca.offset,
                                [bca.ap[0], [0, KG], bca.ap[1]],
                            )
                            nc.vector.tensor_mul(
                                pr[:],
                                E[:, g * KG : (g + 1) * KG, :].bitcast(F32),
                                in1,
                            )
                            nc.sync.dma_start(
                                attn_hb[:, g * KG : (g + 1) * KG, qsl], pr[:]
                            )

                    # out-projection for the s-range of this q strip
                    for st in range(qs * (QSW // 128), (qs + 1) * (QSW // 128)):
                        out_sb = sbA.tile([128, D], F32, tag="out")
                        for nh in range(NOW):
                            po = psA.tile([128, OW], F32, tag="oproj")
                            for h in range(2):
                                nc.tensor.matmul(
                                    po[:],
                                    ctxT_sb[:, h, st * 128 : (st + 1) * 128],
                                    woT_sb[:, h, nh * OW : (nh + 1) * OW],
                                    start=(h == 0),
                                    stop=(h == 1),
                                )
                            nc.scalar.copy(out_sb[:, nh * OW : (nh + 1) * OW], po[:])
                        nc.sync.dma_start(
                            outp_d[b, st * 128 : (st + 1) * 128, :], out_sb[:]
                        )

    nc.compile()
    return nc


_program_cache = {}


def _get_program(B, S, D, repeat=1):
    key = (B, S, D, repeat)
    if key not in _program_cache:
        _program_cache[key] = build_program(B, S, D, repeat)
    return _program_cache[key]


def make_in_maps(x, wq, bq, wk, bk, wv, bv, wo, bo):
    """Host-side shard prep: per-core input dicts."""
    xT = np.ascontiguousarray(x.transpose(0, 2, 1)).astype(np.float32)
    in_maps = []
    for i in range(N_CORES):
        sl = slice(128 * i, 128 * (i + 1))
        wT_i = np.ascontiguousarray(
            np.concatenate([wq[sl].T, wk[sl].T, wv[sl].T], axis=1)
        ).astype(np.float32)
        woT_i = np.ascontiguousarray(
            np.stack(
                [
                    wo[:, 128 * i : 128 * i + 64].T,
                    wo[:, 128 * i + 64 : 128 * i + 128].T,
                ],
                axis=1,
            )
        ).astype(np.float32)
        bqkv_i = np.ascontiguousarray(
            np.stack([bq[sl], bk[sl], bv[sl]], axis=1)
        ).astype(np.float32)
        in_maps.append({"xT": xT, "wT": wT_i, "woT": woT_i, "bqkv": bqkv_i})
    return in_maps


def run(x, wq, bq, wk, bk, wv, bv, wo, bo, B, S, D, trace=False):
    nc = _get_program(B, S, D)
    in_maps = make_in_maps(x, wq, bq, wk, bk, wv, bv, wo, bo)
    res = run_bass_kernel_spmd(nc, in_maps, list(range(N_CORES)), trace=trace)
    out = np.zeros((B, S, D), dtype=np.float32)
    for r in res.results:
        out += r["outp"]
    out += bo.astype(np.float32)
    attnT = np.concatenate([r["attnT"] for r in res.results], axis=0)
    # [H, B, k, q] -> [B, H, q, k] (numpy view; values already computed)
    attn = attnT.transpose(1, 0, 3, 2)
    return (out, attn), res


def kernel(x, wq, bq, wk, bk, wv, bv, wo, bo):
    x = np.asarray(x, dtype=np.float32)
    (out, attn), _ = run(
        x,
        np.asarray(wq, dtype=np.float32),
        np.asarray(bq, dtype=np.float32),
        np.asarray(wk, dtype=np.float32),
        np.asarray(bk, dtype=np.float32),
        np.asarray(wv, dtype=np.float32),
        np.asarray(bv, dtype=np.float32),
        np.asarray(wo, dtype=np.float32),
        np.asarray(bo, dtype=np.float32),
        **FULL_CFG,
    )
    return out, attn


# revision 23
# speedup vs baseline: 10.7407x; 10.7407x over previous
"""Multi-head self-attention (B=4, S=2048, D=1024, H=16) on 8 trn2 NeuronCores.

Sharding: tensor-parallel over heads - each core computes 2 heads (a 128-dim
slice of the q/k/v projection space) for all 4 batches.

Per-core dataflow (all matmuls in float32r - full PE rate, ~2e-4 rel err):
  - host pre-transposes x -> xT [B, D, S] and weight slices.
  - projections produce QT/KT [dims=128, S] directly (contract over D).
  - scores are computed k-major: S^T[k, q] = KT^T-slice @ QT-slice, so the
    exp'd tiles E^T are directly the moving operand of the context matmul
    (contract over k) with V in natural [k, d] layout (from a PE transpose
    of VT).  A ones-column appended to V yields the softmax denominators in
    the same matmul.  exp on ScalarE (<=2ulp); normalization via a
    reciprocal row broadcast across partitions with a K=1 PE matmul and one
    DVE multiply pass.
  - attn_weights are written k-major (attnT [h, b, k, q]); the host returns
    a transposed numpy view (values identical, strides swapped).
  - out-projection partials [B, S, D] are summed on the host (unshard).
"""

import sys

sys.path.insert(0, "/opt/trn_rl_repo")

import numpy as np  # noqa: E402

import concourse.bacc as bacc  # noqa: E402
import concourse.bass as bass  # noqa: E402
import concourse.mybir as mybir  # noqa: E402
import concourse.tile as tile  # noqa: E402
import concourse.tile_utils as tile_utils  # noqa: E402

tile_utils.max_sbuf_usage = 208 * 1024  # 208KB usable on trn2; stale 192 cap

from concourse.bass_utils import run_bass_kernel_spmd  # noqa: E402

F32 = mybir.dt.float32
F32R = mybir.dt.float32r
EXP = mybir.ActivationFunctionType.Exp

N_CORES = 8
HEADS_PER_CORE = 2
HEAD_DIM = 64

FULL_CFG = dict(B=4, S=2048, D=1024)


def build_program(B, S, D, repeat=1):
    """Build the per-core SPMD program (identical on all cores)."""
    NCC = D // 128          # c-chunks (contraction tiles) in d_model
    NKT = S // 128          # k tiles
    QSW = min(512, S)       # q strip width
    NQS = S // QSW          # q strips
    KG = min(2, NKT)        # k-tiles per probs-multiply/DMA group
    NKG = NKT // KG
    OW = min(512, D)        # out-proj N tile width
    NOW = D // OW
    NST = S // 128          # s tiles for out-proj
    scale = 1.0 / np.sqrt(HEAD_DIM)

    nc = bacc.Bacc(None, target_bir_lowering=False, num_swdge_queues=4)

    xT_d = nc.declare_dram_parameter("xT", [B, D, S], F32R, isOutput=False)
    wT_d = nc.declare_dram_parameter("wT", [D, 384], F32R, isOutput=False)
    woT_d = nc.declare_dram_parameter("woT", [64, 2, D], F32R, isOutput=False)
    bqkv_d = nc.declare_dram_parameter("bqkv", [128, 3], F32, isOutput=False)
    attnT_d = nc.declare_dram_parameter(
        "attnT", [HEADS_PER_CORE, B, S, S], F32, isOutput=True
    )
    outp_d = nc.declare_dram_parameter("outp", [B, S, D], F32, isOutput=True)

    with tile.TileContext(nc) as tc:
        with (
            tc.tile_pool(name="cst", bufs=1) as cst,
            tc.tile_pool(name="sbA", bufs=1) as sbA,
            tc.tile_pool(name="sbB", bufs=2) as sbB,
            tc.tile_pool(name="sbC", bufs=2) as sbC,
            tc.tile_pool(name="psA", bufs=1, space="PSUM") as psA,
            tc.tile_pool(name="psB", bufs=2, space="PSUM") as psB,
        ):
            # --- constants ---
            wT_sb = cst.tile([128, NCC, 384], F32R, tag="w")
            nc.sync.dma_start(
                wT_sb[:], wT_d.rearrange("(cc p) w -> p cc w", p=128)
            )
            woT_sb = cst.tile([64, 2, D], F32R, tag="wo")
            nc.sync.dma_start(woT_sb[:], woT_d[:])
            bias_sb = cst.tile([128, 3], F32, tag="b")
            nc.sync.dma_start(bias_sb[:], bqkv_d[:])
            ones_sb = cst.tile([128, 128], F32, tag="ones")
            nc.gpsimd.memset(ones_sb[:], 1.0)
            c32_sb = cst.tile([128, 128], F32, tag="c32")
            nc.gpsimd.memset(c32_sb[:], 1.0 / 8.0)
            # ident64: two stacked 64x64 identities (rows 0-63 and 64-127)
            ident64 = cst.tile([128, 64], F32, tag="id")
            nc.gpsimd.memset(ident64[:], 0.0)
            for hh in range(2):
                nc.gpsimd.affine_select(
                    out=ident64[hh * 64 : (hh + 1) * 64, :],
                    in_=ident64[hh * 64 : (hh + 1) * 64, :],
                    compare_op=mybir.AluOpType.not_equal,
                    fill=1.0,
                    base=0,
                    pattern=[[-1, 64]],
                    channel_multiplier=1,
                )

            for b in [bb for _ in range(repeat) for bb in range(B)]:
                # ---------------- projections for batch b ----------------
                QT_sb = sbC.tile([128, S], F32R, tag="qt")
                KT_sb = sbC.tile([128, S], F32R, tag="kt")
                VT_sb = sbA.tile([128, S], F32, tag="vt")
                V_sb = sbC.tile([128, 2, NKT, 72], F32R, tag="v")
                ctxT_sb = sbA.tile([64, 2, S], F32R, tag="ctx_sb")
                # ones columns (64..71) of each V tile -> softmax denominators
                for h in range(2):
                    ones_in = bass.AP(
                        ones_sb[:].tensor, ones_sb[:].offset,
                        [ones_sb[:].ap[0], [0, NKT], [1, 8]],
                    )
                    nc.vector.tensor_copy(V_sb[:, h, :, 64:72], ones_in)
                xT_b = xT_d[b].rearrange("(cc p) s -> p cc s", p=128)
                for q in range(NQS):
                    xq = sbA.tile([128, NCC, QSW], F32R, tag="xq")
                    nc.sync.dma_start(
                        xq[:], xT_b[:, :, q * QSW : (q + 1) * QSW]
                    )
                    for t, dst in enumerate((QT_sb, KT_sb, VT_sb)):
                        pp = psB.tile([128, QSW], F32, tag="proj")
                        for cc in range(NCC):
                            nc.tensor.matmul(
                                pp[:],
                                wT_sb[:, cc, t * 128 : (t + 1) * 128],
                                xq[:, cc, :],
                                start=(cc == 0),
                                stop=(cc == NCC - 1),
                            )
                        nc.vector.tensor_scalar_add(
                            dst[:, q * QSW : (q + 1) * QSW],
                            pp[:],
                            bias_sb[:, t : t + 1],
                        )
                # V = VT^T per head (cols 0..63 of each V k-tile)
                for h in range(2):
                    for g in range(NKT // 4):
                        tp = psB.tile([128, 8, 64], F32, tag="proj")
                        for j in range(4):
                            kt = g * 4 + j
                            nc.tensor.transpose(
                                tp[:, j, :],
                                VT_sb[h * 64 : (h + 1) * 64, kt * 128 : (kt + 1) * 128],
                                ident64[h * 64 : (h + 1) * 64, :],
                            )
                        nc.vector.tensor_copy(
                            V_sb[:, h, g * 4 : (g + 1) * 4, 0:64], tp[:, 0:4, :]
                        )

                # ------ attention + interleaved out-proj for batch b ------
                for qs in range(NQS):
                    qsl = slice(qs * QSW, (qs + 1) * QSW)
                    for h in range(2):
                        hs = slice(h * 64, (h + 1) * 64)
                        E = sbB.tile([128, NKT, QSW], F32R, tag="e")
                        for kt in range(NKT):
                            sc = psB.tile([128, QSW], F32, tag="score")
                            nc.tensor.matmul(
                                sc[:],
                                KT_sb[hs, kt * 128 : (kt + 1) * 128],
                                QT_sb[hs, qsl],
                                start=True,
                                stop=True,
                            )
                            nc.scalar.activation(
                                E[:, kt, :], sc[:], EXP, bias=0.0, scale=float(scale)
                            )
                        ctx = psB.tile([128, QSW], F32, tag="ctx")
                        for kt in range(NKT):
                            nc.tensor.matmul(
                                ctx[0:72, :],
                                V_sb[:, h, kt, :],
                                E[:, kt, :],
                                start=(kt == 0),
                                stop=(kt == NKT - 1),
                            )
                        # sums row (row 64 of ctx psum) -> partition 0 via
                        # SBUF-SBUF DMA, reciprocal there, then K=1 base-0
                        # matmul broadcasts it across all 128 partitions
                        # (K<64 matmuls at base 64 misread their operands on
                        # HW, so everything here runs at base 0)
                        srow = sbB.tile([65, QSW], F32, tag="sr")
                        nc.vector.tensor_copy(srow[64:65, :], ctx[64:65, :])
                        srow0 = sbB.tile([1, QSW], F32, tag="sr0")
                        nc.sync.dma_start(srow0[0:1, :], srow[64:65, :])
                        rrow = sbB.tile([1, QSW], F32, tag="rc")
                        nc.vector.reciprocal_approx_fast(
                            rrow[0:1, :], srow0[0:1, :]
                        )
                        bc = psA.tile([128, QSW], F32, tag="bcast")
                        nc.tensor.matmul(
                            bc[:],
                            ones_sb[0:1, :],
                            rrow[0:1, :],
                            start=True,
                            stop=True,
                        )
                        bc_sb = sbB.tile([128, QSW], F32, tag="bc")
                        nc.vector.tensor_copy(bc_sb[:], bc[:])
                        # normalized context -> ctxT_sb (f32r for out-proj)
                        nc.vector.tensor_mul(
                            ctxT_sb[:, h, qsl], ctx[0:64, :], bc_sb[0:64, :]
                        )
                        # normalized probs -> DRAM (k-major), in groups of KG
                        attn_hb = attnT_d[h, b].rearrange(
                            "(g p) q -> p g q", p=128
                        )
                        for g in range(NKG):
                            pr = sbB.tile([128, KG, QSW], F32, tag="pr")
                            bca = bc_sb[:]
                            in1 = bass.AP(
                                bca.tensor,
                                bca.offset,
                                [bca.ap[0], [0, KG], bca.ap[1]],
                            )
                            nc.vector.tensor_mul(
                                pr[:],
                                E[:, g * KG : (g + 1) * KG, :].bitcast(F32),
                                in1,
                            )
                            nc.sync.dma_start(
                                attn_hb[:, g * KG : (g + 1) * KG, qsl], pr[:]
                            )

                    # out-projection for the s-range of this q strip
                    for st in range(qs * (QSW // 128), (qs + 1) * (QSW // 128)):
                        out_sb = sbA.tile([128, D], F32, tag="out")
                        for nh in range(NOW):
                            po = psA.tile([128, OW], F32, tag="oproj")
                            for h in range(2):
                                nc.tensor.matmul(
                                    po[:],
                                    ctxT_sb[:, h, st * 128 : (st + 1) * 128],
                                    woT_sb[:, h, nh * OW : (nh + 1) * OW],
                                    start=(h == 0),
                                    stop=(h == 1),
                                )
                            nc.scalar.copy(out_sb[:, nh * OW : (nh + 1) * OW], po[:])
                        nc.sync.dma_start(
                            outp_d[b, st * 128 : (st + 1) * 128, :], out_sb[:]
                        )

    nc.compile()
    return nc


_program_cache = {}


def _get_program(B, S, D, repeat=1):
    key = (B, S, D, repeat)
    if key not in _program_cache:
        _program_cache[key] = build_program(B, S, D, repeat)
    return _program_cache[key]


def make_in_maps(x, wq, bq, wk, bk, wv, bv, wo, bo):
    """Host-side shard prep: per-core input dicts."""
    xT = np.ascontiguousarray(x.transpose(0, 2, 1)).astype(np.float32)
    in_maps = []
    for i in range(N_CORES):
        sl = slice(128 * i, 128 * (i + 1))
        wT_i = np.ascontiguousarray(
            np.concatenate([wq[sl].T, wk[sl].T, wv[sl].T], axis=1)
        ).astype(np.float32)
        woT_i = np.ascontiguousarray(
            np.stack(
                [
                    wo[:, 128 * i : 128 * i + 64].T,
                    wo[:, 128 * i + 64 : 128 * i + 128].T,
                ],
                axis=1,
            )
        ).astype(np.float32)
        bqkv_i = np.ascontiguousarray(
            np.stack([bq[sl], bk[sl], bv[sl]], axis=1)
        ).astype(np.float32)
        in_maps.append({"xT": xT, "wT": wT_i, "woT": woT_i, "bqkv": bqkv_i})
    return in_maps


def run(x, wq, bq, wk, bk, wv, bv, wo, bo, B, S, D, trace=False):
    nc = _get_program(B, S, D)
    in_maps = make_in_maps(x, wq, bq, wk, bk, wv, bv, wo, bo)
    res = run_bass_kernel_spmd(nc, in_maps, list(range(N_CORES)), trace=trace)
    out = np.zeros((B, S, D), dtype=np.float32)
    for r in res.results:
        out += r["outp"]
    out += bo.astype(np.float32)
    attnT = np.concatenate([r["attnT"] for r in res.results], axis=0)
    # [H, B, k, q] -> [B, H, q, k] (numpy view; values already computed)
    attn = attnT.transpose(1, 0, 3, 2)
    return (out, attn), res


def kernel(x, wq, bq, wk, bk, wv, bv, wo, bo):
    x = np.asarray(x, dtype=np.float32)
    (out, attn), _ = run(
        x,
        np.asarray(wq, dtype=np.float32),
        np.asarray(bq, dtype=np.float32),
        np.asarray(wk, dtype=np.float32),
        np.asarray(bk, dtype=np.float32),
        np.asarray(wv, dtype=np.float32),
        np.asarray(bv, dtype=np.float32),
        np.asarray(wo, dtype=np.float32),
        np.asarray(bo, dtype=np.float32),
        **FULL_CFG,
    )
    return out, attn
